# revision 7
# baseline (speedup 1.0000x reference)
"""cosFormer linear-attention transformer forward on 8 TRN2 NeuronCores.

Sharding: token-parallel. Core i handles batch b=i//2, sequence half i%2
(1024 tokens). All position-wise ops (projections, FFN) are local; the
linear-attention kv/ksum statistics are summed across the 2 cores sharing a
batch element with a pairwise AllReduce per layer (~270KB, overlapped with
the q projection).

Layout: activations are feature-major x^T [D(part), T(free)] in SBUF; k/v come
out of the PE token-major, which is exactly what the kv einsum needs as the
stationary operand, so no transposes are needed after the embedding.
Matmul inputs are bf16; PSUM accumulation and the residual stream are fp32.
Wo and W2 are pre-scaled by the ReZero resweight on the host, so residuals
are plain adds.
"""
import math
from contextlib import ExitStack

import numpy as np
import ml_dtypes

import concourse.bass as bass
import concourse.mybir as mybir
import concourse.tile as tile
from concourse import bacc, bass_utils
from concourse.masks import make_identity

F32 = mybir.dt.float32
BF16 = mybir.dt.bfloat16
I32 = mybir.dt.int32
AF = mybir.ActivationFunctionType
ALU = mybir.AluOpType

N_CORES = 8
PAIRS = [[0, 1], [2, 3], [4, 5], [6, 7]]
DH = 64


def build_model_nc(V, D, H, L, FF, T, EPS=1e-5, out_full_x=False):
    """Build the per-core Bass program. T = tokens per core (== D)."""
    KD = D // 128                # feature k-tiles
    TT = T // 128                # token tiles
    NC = min(512, T)             # free-dim chunk for matmuls
    NN = T // NC                 # token chunks
    FT = FF // 128               # ffn row tiles
    FCC = min(1024, FF)          # w1 staging chunk cols
    FC = FF // FCC
    FM = FCC // 128
    VA = DH + 1                  # v augmented with ones column
    assert D == T and D % 128 == 0 and FF % FCC == 0 and H * DH == D
    assert H == 2 * KD

    nc = bacc.Bacc("TRN2", target_bir_lowering=False, debug=False,
                   num_devices=N_CORES)

    ids_d = nc.dram_tensor("ids", [T], I32, kind="ExternalInput")
    emb_d = nc.dram_tensor("emb", [V, D], F32, kind="ExternalInput")
    wq_d = nc.dram_tensor("wq", [L, D, D], BF16, kind="ExternalInput")
    wk_d = nc.dram_tensor("wk", [L, D, D], BF16, kind="ExternalInput")
    wv_d = nc.dram_tensor("wv", [L, D, D], BF16, kind="ExternalInput")
    wo_d = nc.dram_tensor("wo", [L, D, D], BF16, kind="ExternalInput")
    w1_d = nc.dram_tensor("w1", [L, D, FF], BF16, kind="ExternalInput")
    w2_d = nc.dram_tensor("w2", [L, FF, D], BF16, kind="ExternalInput")
    cwsw_d = nc.dram_tensor("cwsw", [128, T], BF16, kind="ExternalInput")
    swcw_d = nc.dram_tensor("swcw", [128, T], BF16, kind="ExternalInput")
    kcw_d = nc.dram_tensor("kcw", [128, TT], F32, kind="ExternalInput")
    ksw_d = nc.dram_tensor("ksw", [128, TT], F32, kind="ExternalInput")
    x0_d = nc.dram_tensor("x0", [KD, 128], F32, kind="ExternalOutput")
    if out_full_x:
        xf_d = nc.dram_tensor("xfull", [KD, 128, T], F32,
                              kind="ExternalOutput")

    with tile.TileContext(nc) as tc, ExitStack() as ctx:
        const = ctx.enter_context(tc.tile_pool(name="const", bufs=1))
        persist = ctx.enter_context(tc.tile_pool(name="persist", bufs=1))
        small = ctx.enter_context(tc.tile_pool(name="small", bufs=2))
        wstr = ctx.enter_context(tc.tile_pool(name="wstr", bufs=3))
        drp = ctx.enter_context(tc.tile_pool(name="drp", bufs=2, space="DRAM"))

        # ---- constants ----
        cwsw = const.tile([128, T], BF16, tag="cwsw")
        nc.sync.dma_start(out=cwsw[:, :], in_=cwsw_d[:, :])
        swcw = const.tile([128, T], BF16, tag="swcw")
        nc.sync.dma_start(out=swcw[:, :], in_=swcw_d[:, :])
        kcw = const.tile([128, TT], F32, tag="kcw")
        ksw = const.tile([128, TT], F32, tag="ksw")
        nc.sync.dma_start(out=kcw[:, :], in_=kcw_d[:, :])
        nc.sync.dma_start(out=ksw[:, :], in_=ksw_d[:, :])
        ones_bf = const.tile([128, DH], BF16, tag="ones")
        nc.vector.memset(ones_bf[:, :], 1.0)
        ident = const.tile([128, 128], F32, tag="ident")
        make_identity(nc, ident)

        xT = persist.tile([128, KD, T], F32, tag="xT")
        xbf = persist.tile([128, KD, T], BF16, tag="xbf")

        # ---- embedding gather + transpose to feature-major ----
        ids_sb = const.tile([128, TT], I32, tag="ids")
        nc.sync.dma_start(
            out=ids_sb[:, :],
            in_=bass.AP(tensor=ids_d, offset=0, ap=[[1, 128], [128, TT]]),
        )
        with tc.tile_pool(name="emb", bufs=2) as ep, \
             tc.tile_pool(name="embp", bufs=3, space="PSUM") as epp:
            for t in range(TT):
                xtok = ep.tile([128, D], F32, tag="xtok")
                nc.gpsimd.indirect_dma_start(
                    out=xtok[:, :], out_offset=None,
                    in_=emb_d[:, :],
                    in_offset=bass.IndirectOffsetOnAxis(
                        ap=ids_sb[:, t:t + 1], axis=0),
                )
                for k in range(KD):
                    ps = epp.tile([128, 128], F32, tag="tp")
                    nc.tensor.transpose(ps[:, :],
                                        xtok[:, k * 128:(k + 1) * 128],
                                        ident[:, :])
                    nc.scalar.activation(xT[:, k, t * 128:(t + 1) * 128],
                                         ps[:, :], AF.Copy)
                    nc.vector.tensor_copy(xbf[:, k, t * 128:(t + 1) * 128],
                                          ps[:, :])

        # ---- layers ----
        for l in range(L):
            # ---------- k/v projections + kv aggregation ----------
            kvacc = persist.tile([128, H * VA], F32, tag="kvacc")
            nc.vector.memset(kvacc[:, :], 0.0)
            with tc.tile_pool(name=f"wkv{l}", bufs=1) as wkvp, \
                 tc.tile_pool(name=f"kvt{l}", bufs=2) as kvtp, \
                 tc.tile_pool(name=f"pkv{l}", bufs=2, space="PSUM") as pkvp, \
                 tc.tile_pool(name=f"pag{l}", bufs=3, space="PSUM") as pagp:
                wk_s = wkvp.tile([128, KD, D], BF16, tag="wk")
                wv_s = wkvp.tile([128, KD, D], BF16, tag="wv")
                for wsb, wdr in ((wk_s, wk_d), (wv_s, wv_d)):
                    nc.sync.dma_start(
                        out=wsb[:, :, :],
                        in_=bass.AP(tensor=wdr, offset=l * D * D,
                                    ap=[[D, 128], [128 * D, KD], [1, D]]),
                    )
                for t in range(TT):
                    kft = kvtp.tile([128, 2 * D], BF16, tag="kft")
                    vat = kvtp.tile([128, H * VA], BF16, tag="vat")
                    va4 = vat[:, :].rearrange("p (h c) -> p h c", c=VA)
                    nc.vector.memset(va4[:, :, DH:DH + 1], 1.0)
                    kf4 = kft[:, :].rearrange("p (h s c) -> p h s c",
                                              h=H, s=2)
                    for n in range(D // NC):
                        esl = bass.ts(n, NC)
                        hpc = NC // DH
                        h0 = n * NC // DH
                        psk = pkvp.tile([128, NC], F32, tag="psk")
                        psv = pkvp.tile([128, NC], F32, tag="psv")
                        for k in range(KD):
                            lhs = xbf[:, k, t * 128:(t + 1) * 128]
                            nc.tensor.matmul(psk[:, :], lhs, wk_s[:, k, esl],
                                             start=(k == 0),
                                             stop=(k == KD - 1))
                        for k in range(KD):
                            lhs = xbf[:, k, t * 128:(t + 1) * 128]
                            nc.tensor.matmul(psv[:, :], lhs, wv_s[:, k, esl],
                                             start=(k == 0),
                                             stop=(k == KD - 1))
                        psk4 = psk[:, :].rearrange("p (h c) -> p h c", c=DH)
                        psv4 = psv[:, :].rearrange("p (h c) -> p h c", c=DH)
                        nc.scalar.activation(kf4[:, h0:h0 + hpc, 0, :],
                                             psk4[:, :, :], AF.Relu,
                                             scale=kcw[:, t:t + 1])
                        nc.scalar.activation(kf4[:, h0:h0 + hpc, 1, :],
                                             psk4[:, :, :], AF.Relu,
                                             scale=ksw[:, t:t + 1])
                        nc.vector.tensor_copy(va4[:, h0:h0 + hpc, 0:DH],
                                              psv4[:, :, :])
                    for h in range(H):
                        pkv = pagp.tile([128, VA], F32, tag="pkv")
                        nc.tensor.matmul(pkv[:, :],
                                         kft[:, h * 128:(h + 1) * 128],
                                         vat[:, h * VA:(h + 1) * VA],
                                         start=True, stop=True)
                        nc.vector.tensor_add(kvacc[:, h * VA:(h + 1) * VA],
                                             kvacc[:, h * VA:(h + 1) * VA],
                                             pkv[:, :])

            # ---------- AllReduce kv stats across the pair ----------
            kvloc = persist.tile([128, H * VA], BF16, tag="kvloc")
            nc.vector.tensor_copy(kvloc[:, :], kvacc[:, :])
            cc_in = drp.tile([128, H * VA], BF16, tag="ccin")
            cc_out = drp.tile([128, H * VA], BF16, tag="ccout")
            nc.sync.dma_start(out=cc_in[:, :], in_=kvloc[:, :])
            nc.gpsimd.collective_compute(
                "AllReduce", ALU.add, replica_groups=PAIRS,
                ins=[cc_in[:, :]], outs=[cc_out[:, :]])
            kvg = persist.tile([128, H, VA], BF16, tag="kvg")
            nc.sync.dma_start(out=kvg[:, :, :], in_=cc_out[:, :])

            # ---------- q projection / attention / Wo ----------
            qt = persist.tile([128, KD, T], BF16, tag="qt")
            attnT = persist.tile([128, KD, T], BF16, tag="attnT")
            with tc.tile_pool(name=f"wqo{l}", bufs=1) as wqop, \
                 tc.tile_pool(name=f"pq{l}", bufs=3, space="PSUM") as pqp:
                wq_s = wqop.tile([128, KD, D], BF16, tag="wq")
                wo_s = wqop.tile([128, KD, D], BF16, tag="wo")
                for wsb, wdr in ((wq_s, wq_d), (wo_s, wo_d)):
                    nc.sync.dma_start(
                        out=wsb[:, :, :],
                        in_=bass.AP(tensor=wdr, offset=l * D * D,
                                    ap=[[D, 128], [128 * D, KD], [1, D]]),
                    )
                for m in range(KD):
                    for n in range(NN):
                        nsl = bass.ts(n, NC)
                        psq = pqp.tile([128, NC], F32, tag="psq", bufs=2)
                        for k in range(KD):
                            nc.tensor.matmul(
                                psq[:, :], wq_s[:, k, m * 128:(m + 1) * 128],
                                xbf[:, k, nsl],
                                start=(k == 0), stop=(k == KD - 1))
                        nc.scalar.activation(qt[:, m, nsl], psq[:, :], AF.Relu)

                for h in range(H):
                    j, half = h // 2, h % 2
                    qh = qt[half * 64:half * 64 + 64, j, :]
                    qf = small.tile([128, T], BF16, tag="qf")
                    # SBUF tensor_tensor inputs must share a base partition:
                    # cwsw=[cw;sw], swcw=[sw;cw] so each half reads in place.
                    hsl = slice(half * 64, half * 64 + 64)
                    cwh = cwsw[hsl, :] if half == 0 else swcw[hsl, :]
                    swh = swcw[hsl, :] if half == 0 else cwsw[hsl, :]
                    nc.vector.tensor_mul(qf[0:64, :], qh, cwh)
                    nc.vector.tensor_mul(qf[64:128, :], qh, swh)
                    for n in range(NN):
                        nsl = bass.ts(n, NC)
                        pso = pqp.tile([128, NC], F32, tag="pso")
                        nc.tensor.matmul(pso[0:VA, :], kvg[:, h, :],
                                         qf[:, nsl], start=True, stop=True)
                        zf = small.tile([128, NC], F32, tag="zf")
                        zb = small.tile([128, NC], BF16, tag="zb")
                        nc.vector.tensor_scalar(zf[DH:DH + 1, :],
                                                pso[DH:DH + 1, :], EPS, None,
                                                op0=ALU.add)
                        with nc.allow_low_precision(
                                reason="z broadcast via PE needs bf16 rhs"):
                            nc.vector.reciprocal(zb[DH:DH + 1, :],
                                                 zf[DH:DH + 1, :])
                        psz = pqp.tile([128, NC], F32, tag="psz", bufs=2)
                        nc.tensor.matmul(psz[0:64, :], ones_bf[DH:DH + 1, :],
                                         zb[DH:DH + 1, :],
                                         start=True, stop=True)
                        nb = small.tile([128, NC], BF16, tag="nb")
                        nc.scalar.activation(nb[0:64, :], pso[0:64, :],
                                             AF.Copy)
                        nc.vector.tensor_mul(
                            attnT[half * 64:half * 64 + 64, j, nsl],
                            nb[0:64, :], psz[0:64, :])

                for m in range(KD):
                    for n in range(NN):
                        nsl = bass.ts(n, NC)
                        pswo = pqp.tile([128, NC], F32, tag="pso")
                        for k in range(KD):
                            nc.tensor.matmul(
                                pswo[:, :], wo_s[:, k, m * 128:(m + 1) * 128],
                                attnT[:, k, nsl],
                                start=(k == 0), stop=(k == KD - 1))
                        nc.vector.tensor_add(xT[:, m, nsl], xT[:, m, nsl],
                                             pswo[:, :])
                        nc.vector.tensor_copy(xbf[:, m, nsl], xT[:, m, nsl])

            # ---------- FFN ----------
            with tc.tile_pool(name=f"ffn{l}", bufs=1) as ffp:
                hsb = ffp.tile([128, FT, NC], BF16, tag="hsb")
                for n in range(NN):
                    nsl = bass.ts(n, NC)
                    with tc.tile_pool(name=f"ph{l}_{n}", bufs=3,
                                      space="PSUM") as php:
                        for c in range(FC):
                            w1c = wstr.tile([128, KD, FCC], BF16, tag="w1c")
                            nc.sync.dma_start(
                                out=w1c[:, :, :],
                                in_=bass.AP(tensor=w1_d,
                                            offset=l * D * FF + c * FCC,
                                            ap=[[FF, 128], [128 * FF, KD],
                                                [1, FCC]]),
                            )
                            for m in range(FM):
                                psh = php.tile([128, NC], F32, tag="psh")
                                for k in range(KD):
                                    nc.tensor.matmul(
                                        psh[:, :],
                                        w1c[:, k, m * 128:(m + 1) * 128],
                                        xbf[:, k, nsl],
                                        start=(k == 0), stop=(k == KD - 1))
                                nc.scalar.activation(hsb[:, c * FM + m, :],
                                                     psh[:, :], AF.Relu)
                    with tc.tile_pool(name=f"pw{l}_{n}", bufs=1,
                                      space="PSUM") as psW:
                        pso2 = []
                        for e in range(KD):
                            po = psW.tile([128, NC], F32, tag=f"po{e}",
                                          name=f"po{l}_{n}_{e}")
                            pso2.append(po)
                        for ff in range(FT):
                            w2t = wstr.tile([128, D], BF16, tag="w2t")
                            nc.sync.dma_start(
                                out=w2t[:, :],
                                in_=w2_d[l, ff * 128:(ff + 1) * 128, :])
                            for e in range(KD):
                                nc.tensor.matmul(
                                    pso2[e][:, :],
                                    w2t[:, e * 128:(e + 1) * 128],
                                    hsb[:, ff, :],
                                    start=(ff == 0), stop=(ff == FT - 1))
                        for e in range(KD):
                            nc.vector.tensor_add(xT[:, e, nsl], xT[:, e, nsl],
                                                 pso2[e][:, :])
                            nc.vector.tensor_copy(xbf[:, e, nsl],
                                                  xT[:, e, nsl])

        # ---- outputs ----
        for k in range(KD):
            nc.sync.dma_start(out=x0_d[k, :], in_=xT[:, k, 0:1])
        if out_full_x:
            for k in range(KD):
                nc.sync.dma_start(out=xf_d[k, :, :], in_=xT[:, k, :])

    nc.compile()
    return nc


_NC_CACHE = {}


def get_nc(key, *args, **kw):
    if key not in _NC_CACHE:
        _NC_CACHE[key] = build_model_nc(*args, **kw)
    return _NC_CACHE[key]


def make_in_maps(input_ids, attention_mask, lengths, emb_in, Wq, Wk, Wv, Wo,
                 W1, W2, resweight, S, T):
    ids = np.asarray(input_ids).astype(np.int32)
    mask = np.asarray(attention_mask).astype(np.float32)
    lens = np.asarray(lengths).astype(np.float32)
    emb = np.ascontiguousarray(np.asarray(emb_in, dtype=np.float32))
    rw = np.asarray(resweight, dtype=np.float32)

    bf = ml_dtypes.bfloat16
    wq = np.ascontiguousarray(np.asarray(Wq, dtype=np.float32)).astype(bf)
    wk = np.ascontiguousarray(np.asarray(Wk, dtype=np.float32)).astype(bf)
    wv = np.ascontiguousarray(np.asarray(Wv, dtype=np.float32)).astype(bf)
    wo = np.ascontiguousarray(
        np.asarray(Wo, dtype=np.float32) * rw[:, None, None]).astype(bf)
    w1 = np.ascontiguousarray(np.asarray(W1, dtype=np.float32)).astype(bf)
    w2 = np.ascontiguousarray(
        np.asarray(W2, dtype=np.float32) * rw[:, None, None]).astype(bf)

    s_idx = np.arange(S, dtype=np.float32)
    ang = (math.pi / 2) * s_idx[None, :] / lens[:, None]
    cw, sw = np.cos(ang), np.sin(ang)
    kcw_f, ksw_f = cw * mask, sw * mask

    in_maps = []
    for c in range(N_CORES):
        b, half = c // 2, c % 2
        sl = slice(half * T, (half + 1) * T)
        cwsw = np.empty((128, T), np.float32)
        cwsw[0:64, :] = cw[b, sl][None, :]
        cwsw[64:128, :] = sw[b, sl][None, :]
        swcw = np.empty((128, T), np.float32)
        swcw[0:64, :] = sw[b, sl][None, :]
        swcw[64:128, :] = cw[b, sl][None, :]
        in_maps.append({
            "ids": np.ascontiguousarray(ids[b, sl]),
            "emb": emb,
            "wq": wq, "wk": wk, "wv": wv, "wo": wo, "w1": w1, "w2": w2,
            "cwsw": cwsw.astype(bf),
            "swcw": swcw.astype(bf),
            "kcw": np.ascontiguousarray(
                kcw_f[b, sl].reshape(T // 128, 128).T).astype(np.float32),
            "ksw": np.ascontiguousarray(
                ksw_f[b, sl].reshape(T // 128, 128).T).astype(np.float32),
        })
    return in_maps


def head_loss(res_list, emb_out, labels, B, C):
    eo = np.asarray(emb_out, dtype=np.float32)
    lab = np.asarray(labels).astype(np.int64)
    x0 = np.stack([np.asarray(res_list[2 * b]["x0"]).reshape(-1)
                   for b in range(B)])
    logits = x0 @ eo
    m = logits.max(axis=1, keepdims=True)
    logp = logits - (m + np.log(np.exp(logits - m).sum(axis=1, keepdims=True)))
    loss = np.float32(-np.mean(logp[np.arange(B), lab]))
    return loss, logits.astype(np.float32)


def kernel(input_ids, labels, attention_mask, lengths, emb_in, Wq, Wk, Wv, Wo,
           W1, W2, resweight, emb_out):
    B, S, D, H, L, V, C = 4, 2048, 1024, 16, 6, 32000, 4
    FF, T = 4 * D, 1024

    in_maps = make_in_maps(input_ids, attention_mask, lengths, emb_in,
                           Wq, Wk, Wv, Wo, W1, W2, resweight, S, T)
    nc = get_nc("full", V, D, H, L, FF, T)
    res = bass_utils.run_bass_kernel_spmd(nc, in_maps,
                                          core_ids=list(range(N_CORES)))
    return head_loss(res.results, emb_out, labels, B, C)


# revision 9
# speedup vs baseline: 1.0599x; 1.0599x over previous
"""cosFormer linear-attention transformer forward on 8 TRN2 NeuronCores.

Sharding: token-parallel. Core i handles batch b=i//2, sequence half i%2
(1024 tokens). All position-wise ops (projections, FFN) are local; the
linear-attention kv/ksum statistics are summed across the 2 cores sharing a
batch element with a pairwise AllReduce per layer (~270KB, overlapped with
the q projection).

Layout: activations are feature-major x^T [D(part), T(free)] in SBUF; k/v come
out of the PE token-major, which is exactly what the kv einsum needs as the
stationary operand, so no transposes are needed after the embedding.
Matmul inputs are bf16; PSUM accumulation and the residual stream are fp32.
Wo and W2 are pre-scaled by the ReZero resweight on the host, so residuals
are plain adds.
"""
import math
from contextlib import ExitStack

import numpy as np
import ml_dtypes

import concourse.bass as bass
import concourse.mybir as mybir
import concourse.tile as tile
from concourse import bacc, bass_utils
from concourse.masks import make_identity

F32 = mybir.dt.float32
BF16 = mybir.dt.bfloat16
I32 = mybir.dt.int32
AF = mybir.ActivationFunctionType
ALU = mybir.AluOpType

N_CORES = 8
PAIRS = [[0, 1], [2, 3], [4, 5], [6, 7]]
DH = 64


def build_model_nc(V, D, H, L, FF, T, EPS=1e-5, out_full_x=False):
    """Build the per-core Bass program. T = tokens per core (== D)."""
    KD = D // 128                # feature k-tiles
    TT = T // 128                # token tiles
    NC = min(512, T)             # free-dim chunk for matmuls
    NN = T // NC                 # token chunks
    FT = FF // 128               # ffn row tiles
    FCC = min(512, FF)           # w1 staging chunk cols
    FC = FF // FCC
    FM = FCC // 128
    VA = DH + 1                  # v augmented with ones column
    assert D == T and D % 128 == 0 and FF % FCC == 0 and H * DH == D
    assert H == 2 * KD

    nc = bacc.Bacc("TRN2", target_bir_lowering=False, debug=False,
                   num_devices=N_CORES)

    ids_d = nc.dram_tensor("ids", [T], I32, kind="ExternalInput")
    emb_d = nc.dram_tensor("emb", [V, D], F32, kind="ExternalInput")
    wq_d = nc.dram_tensor("wq", [L, D, D], BF16, kind="ExternalInput")
    wk_d = nc.dram_tensor("wk", [L, D, D], BF16, kind="ExternalInput")
    wv_d = nc.dram_tensor("wv", [L, D, D], BF16, kind="ExternalInput")
    wo_d = nc.dram_tensor("wo", [L, D, D], BF16, kind="ExternalInput")
    w1_d = nc.dram_tensor("w1", [L, D, FF], BF16, kind="ExternalInput")
    w2_d = nc.dram_tensor("w2", [L, FF, D], BF16, kind="ExternalInput")
    cwsw_d = nc.dram_tensor("cwsw", [128, T], BF16, kind="ExternalInput")
    swcw_d = nc.dram_tensor("swcw", [128, T], BF16, kind="ExternalInput")
    kcw_d = nc.dram_tensor("kcw", [128, TT], F32, kind="ExternalInput")
    ksw_d = nc.dram_tensor("ksw", [128, TT], F32, kind="ExternalInput")
    x0_d = nc.dram_tensor("x0", [KD, 128], F32, kind="ExternalOutput")
    if out_full_x:
        xf_d = nc.dram_tensor("xfull", [KD, 128, T], F32,
                              kind="ExternalOutput")

    with tile.TileContext(nc) as tc, ExitStack() as ctx:
        const = ctx.enter_context(tc.tile_pool(name="const", bufs=1))
        persist = ctx.enter_context(tc.tile_pool(name="persist", bufs=1))
        small = ctx.enter_context(tc.tile_pool(name="small", bufs=2))
        wstr = ctx.enter_context(tc.tile_pool(name="wstr", bufs=3))
        drp = ctx.enter_context(tc.tile_pool(name="drp", bufs=2, space="DRAM"))

        # ---- constants ----
        cwsw = const.tile([128, T], BF16, tag="cwsw")
        nc.sync.dma_start(out=cwsw[:, :], in_=cwsw_d[:, :])
        swcw = const.tile([128, T], BF16, tag="swcw")
        nc.sync.dma_start(out=swcw[:, :], in_=swcw_d[:, :])
        kcw = const.tile([128, TT], F32, tag="kcw")
        ksw = const.tile([128, TT], F32, tag="ksw")
        nc.sync.dma_start(out=kcw[:, :], in_=kcw_d[:, :])
        nc.sync.dma_start(out=ksw[:, :], in_=ksw_d[:, :])
        ones_bf = const.tile([128, DH], BF16, tag="ones")
        nc.vector.memset(ones_bf[:, :], 1.0)
        epsrow = const.tile([128, NC], BF16, tag="epsrow")
        nc.vector.memset(epsrow[:, :], EPS)
        ident = const.tile([128, 128], F32, tag="ident")
        make_identity(nc, ident)

        xT = persist.tile([128, KD, T], F32, tag="xT")
        xbf = persist.tile([128, KD, T], BF16, tag="xbf")

        # ---- embedding gather + transpose to feature-major ----
        ids_sb = const.tile([128, TT], I32, tag="ids")
        nc.sync.dma_start(
            out=ids_sb[:, :],
            in_=bass.AP(tensor=ids_d, offset=0, ap=[[1, 128], [128, TT]]),
        )
        with tc.tile_pool(name="emb", bufs=2) as ep, \
             tc.tile_pool(name="embp", bufs=3, space="PSUM") as epp:
            for t in range(TT):
                xtok = ep.tile([128, D], F32, tag="xtok")
                nc.gpsimd.indirect_dma_start(
                    out=xtok[:, :], out_offset=None,
                    in_=emb_d[:, :],
                    in_offset=bass.IndirectOffsetOnAxis(
                        ap=ids_sb[:, t:t + 1], axis=0),
                )
                for k in range(KD):
                    ps = epp.tile([128, 128], F32, tag="tp")
                    nc.tensor.transpose(ps[:, :],
                                        xtok[:, k * 128:(k + 1) * 128],
                                        ident[:, :])
                    nc.scalar.activation(xT[:, k, t * 128:(t + 1) * 128],
                                         ps[:, :], AF.Copy)
                    nc.vector.tensor_copy(xbf[:, k, t * 128:(t + 1) * 128],
                                          ps[:, :])

        # ---- layers ----
        for l in range(L):
            # ---------- k/v projections + kv aggregation ----------
            # kv accumulated in PSUM: heads grouped 7+7+2 per bank tile
            HG = [list(range(0, min(7, H)))]
            if H > 7:
                HG.append(list(range(7, min(14, H))))
            if H > 14:
                HG.append(list(range(14, H)))
            with tc.tile_pool(name=f"wkv{l}", bufs=1) as wkvp, \
                 tc.tile_pool(name=f"kvt{l}", bufs=2) as kvtp, \
                 tc.tile_pool(name=f"pkv{l}", bufs=2, space="PSUM") as pkvp, \
                 tc.tile_pool(name=f"pag{l}", bufs=1, space="PSUM") as pagp:
                pag = []
                for g, hs in enumerate(HG):
                    pg = pagp.tile([128, len(hs) * VA], F32, tag=f"pag{g}",
                                   name=f"pag{l}_{g}")
                    pag.append(pg)
                wk_s = wkvp.tile([128, KD, D], BF16, tag="wk")
                wv_s = wkvp.tile([128, KD, D], BF16, tag="wv")
                for wsb, wdr in ((wk_s, wk_d), (wv_s, wv_d)):
                    for k in range(KD):
                        nc.sync.dma_start(
                            out=wsb[:, k, :],
                            in_=bass.AP(tensor=wdr,
                                        offset=l * D * D + k * 128 * D,
                                        ap=[[D, 128], [1, D]]),
                        )
                for t in range(TT):
                    kft = kvtp.tile([128, 2 * D], BF16, tag="kft")
                    vat = kvtp.tile([128, H * VA], BF16, tag="vat")
                    va4 = vat[:, :].rearrange("p (h c) -> p h c", c=VA)
                    nc.vector.memset(va4[:, :, DH:DH + 1], 1.0)
                    kf4 = kft[:, :].rearrange("p (h s c) -> p h s c",
                                              h=H, s=2)
                    for n in range(D // NC):
                        esl = bass.ts(n, NC)
                        hpc = NC // DH
                        h0 = n * NC // DH
                        psk = pkvp.tile([128, NC], F32, tag="psk")
                        psv = pkvp.tile([128, NC], F32, tag="psv")
                        for k in range(KD):
                            lhs = xbf[:, k, t * 128:(t + 1) * 128]
                            nc.tensor.matmul(psk[:, :], lhs, wk_s[:, k, esl],
                                             start=(k == 0),
                                             stop=(k == KD - 1))
                        for k in range(KD):
                            lhs = xbf[:, k, t * 128:(t + 1) * 128]
                            nc.tensor.matmul(psv[:, :], lhs, wv_s[:, k, esl],
                                             start=(k == 0),
                                             stop=(k == KD - 1))
                        psk4 = psk[:, :].rearrange("p (h c) -> p h c", c=DH)
                        psv4 = psv[:, :].rearrange("p (h c) -> p h c", c=DH)
                        nc.scalar.activation(kf4[:, h0:h0 + hpc, 0, :],
                                             psk4[:, :, :], AF.Relu,
                                             scale=kcw[:, t:t + 1])
                        nc.scalar.activation(kf4[:, h0:h0 + hpc, 1, :],
                                             psk4[:, :, :], AF.Relu,
                                             scale=ksw[:, t:t + 1])
                        nc.scalar.activation(va4[:, h0:h0 + hpc, 0:DH],
                                             psv4[:, :, :], AF.Copy)
                    for g, hs in enumerate(HG):
                        for i, h in enumerate(hs):
                            nc.tensor.matmul(
                                pag[g][:, i * VA:(i + 1) * VA],
                                kft[:, h * 128:(h + 1) * 128],
                                vat[:, h * VA:(h + 1) * VA],
                                start=(t == 0 and i == 0),
                                stop=(t == TT - 1 and i == len(hs) - 1))

                # copy PSUM groups -> kvloc bf16 (inside kv pools scope)
                kvloc = persist.tile([128, H * VA], BF16, tag="kvloc")
                off = 0
                for g, hs in enumerate(HG):
                    w = len(hs) * VA
                    nc.vector.tensor_copy(kvloc[:, off:off + w], pag[g][:, :])
                    off += w

            # ---------- AllReduce kv stats across the pair ----------
            cc_in = drp.tile([128, H * VA], BF16, tag="ccin")
            cc_out = drp.tile([128, H * VA], BF16, tag="ccout")
            nc.sync.dma_start(out=cc_in[:, :], in_=kvloc[:, :])
            nc.gpsimd.collective_compute(
                "AllReduce", ALU.add, replica_groups=PAIRS,
                ins=[cc_in[:, :]], outs=[cc_out[:, :]])
            kvg = persist.tile([128, H, VA], BF16, tag="kvg")
            nc.sync.dma_start(out=kvg[:, :, :], in_=cc_out[:, :])

            # ---------- q projection / attention / Wo ----------
            qt = persist.tile([128, KD, T], BF16, tag="qt")
            attnT = persist.tile([128, KD, T], BF16, tag="attnT")
            with tc.tile_pool(name=f"wqo{l}", bufs=1) as wqop, \
                 tc.tile_pool(name=f"pq{l}", bufs=3, space="PSUM") as pqp:
                wq_s = wqop.tile([128, KD, D], BF16, tag="wq")
                wo_s = wqop.tile([128, KD, D], BF16, tag="wo")
                for wsb, wdr in ((wq_s, wq_d), (wo_s, wo_d)):
                    for k in range(KD):
                        nc.sync.dma_start(
                            out=wsb[:, k, :],
                            in_=bass.AP(tensor=wdr,
                                        offset=l * D * D + k * 128 * D,
                                        ap=[[D, 128], [1, D]]),
                        )
                for m in range(KD):
                    for n in range(NN):
                        nsl = bass.ts(n, NC)
                        psq = pqp.tile([128, NC], F32, tag="psq", bufs=2)
                        for k in range(KD):
                            nc.tensor.matmul(
                                psq[:, :], wq_s[:, k, m * 128:(m + 1) * 128],
                                xbf[:, k, nsl],
                                start=(k == 0), stop=(k == KD - 1))
                        nc.scalar.activation(qt[:, m, nsl], psq[:, :], AF.Relu)

                for h in range(H):
                    j, half = h // 2, h % 2
                    qh = qt[half * 64:half * 64 + 64, j, :]
                    qf = small.tile([128, T], BF16, tag="qf")
                    # SBUF tensor_tensor inputs must share a base partition:
                    # cwsw=[cw;sw], swcw=[sw;cw] so each half reads in place.
                    hsl = slice(half * 64, half * 64 + 64)
                    cwh = cwsw[hsl, :] if half == 0 else swcw[hsl, :]
                    swh = swcw[hsl, :] if half == 0 else cwsw[hsl, :]
                    nc.gpsimd.tensor_mul(qf[0:64, :], qh, cwh)
                    nc.gpsimd.tensor_mul(qf[64:128, :], qh, swh)
                    for n in range(NN):
                        nsl = bass.ts(n, NC)
                        pso = pqp.tile([128, NC], F32, tag="pso")
                        nc.tensor.matmul(pso[0:VA, :], kvg[:, h, :],
                                         qf[:, nsl], start=True, stop=True)
                        zf = small.tile([128, NC], F32, tag="zf")
                        zb = small.tile([128, NC], BF16, tag="zb")
                        nc.vector.tensor_scalar(zf[DH:DH + 1, :],
                                                pso[DH:DH + 1, :], EPS, None,
                                                op0=ALU.add)
                        with nc.allow_low_precision(
                                reason="z broadcast via PE needs bf16 rhs"):
                            nc.vector.reciprocal(zb[DH:DH + 1, :],
                                                 zf[DH:DH + 1, :])
                        psz = pqp.tile([128, NC], F32, tag="psz", bufs=2)
                        nc.tensor.matmul(psz[0:64, :], ones_bf[DH:DH + 1, :],
                                         zb[DH:DH + 1, :],
                                         start=True, stop=True)
                        nb = small.tile([128, NC], BF16, tag="nb")
                        nc.scalar.activation(nb[0:64, :], pso[0:64, :],
                                             AF.Copy)
                        nc.vector.tensor_mul(
                            attnT[half * 64:half * 64 + 64, j, nsl],
                            nb[0:64, :], psz[0:64, :])

                for m in range(KD):
                    for n in range(NN):
                        nsl = bass.ts(n, NC)
                        pswo = pqp.tile([128, NC], F32, tag="pso")
                        for k in range(KD):
                            nc.tensor.matmul(
                                pswo[:, :], wo_s[:, k, m * 128:(m + 1) * 128],
                                attnT[:, k, nsl],
                                start=(k == 0), stop=(k == KD - 1))
                        nc.vector.tensor_add(xT[:, m, nsl], xT[:, m, nsl],
                                             pswo[:, :])
                        nc.gpsimd.tensor_copy(xbf[:, m, nsl], xT[:, m, nsl])

            # ---------- FFN ----------
            with tc.tile_pool(name=f"ffn{l}", bufs=1) as ffp:
                hsb = ffp.tile([128, FT, NC], BF16, tag="hsb")
                for n in range(NN):
                    nsl = bass.ts(n, NC)
                    with tc.tile_pool(name=f"ph{l}_{n}", bufs=3,
                                      space="PSUM") as php:
                        for c in range(FC):
                            w1c = wstr.tile([128, KD, FCC], BF16, tag="w1c", bufs=4)
                            nc.sync.dma_start(
                                out=w1c[:, :, :],
                                in_=bass.AP(tensor=w1_d,
                                            offset=l * D * FF + c * FCC,
                                            ap=[[FF, 128], [128 * FF, KD],
                                                [1, FCC]]),
                            )
                            for m in range(FM):
                                psh = php.tile([128, NC], F32, tag="psh")
                                for k in range(KD):
                                    nc.tensor.matmul(
                                        psh[:, :],
                                        w1c[:, k, m * 128:(m + 1) * 128],
                                        xbf[:, k, nsl],
                                        start=(k == 0), stop=(k == KD - 1))
                                nc.scalar.activation(hsb[:, c * FM + m, :],
                                                     psh[:, :], AF.Relu)
                    with tc.tile_pool(name=f"pw{l}_{n}", bufs=1,
                                      space="PSUM") as psW:
                        pso2 = []
                        for e in range(KD):
                            po = psW.tile([128, NC], F32, tag=f"po{e}",
                                          name=f"po{l}_{n}_{e}")
                            pso2.append(po)
                        for ff in range(FT):
                            w2t = wstr.tile([128, D], BF16, tag="w2t", bufs=6)
                            nc.sync.dma_start(
                                out=w2t[:, :],
                                in_=w2_d[l, ff * 128:(ff + 1) * 128, :])
                            for e in range(KD):
                                nc.tensor.matmul(
                                    pso2[e][:, :],
                                    w2t[:, e * 128:(e + 1) * 128],
                                    hsb[:, ff, :],
                                    start=(ff == 0), stop=(ff == FT - 1))
                        for e in range(KD):
                            nc.vector.tensor_add(xT[:, e, nsl], xT[:, e, nsl],
                                                 pso2[e][:, :])
                            nc.gpsimd.tensor_copy(xbf[:, e, nsl],
                                                  xT[:, e, nsl])

        # ---- outputs ----
        for k in range(KD):
            nc.sync.dma_start(out=x0_d[k, :], in_=xT[:, k, 0:1])
        if out_full_x:
            for k in range(KD):
                nc.sync.dma_start(out=xf_d[k, :, :], in_=xT[:, k, :])

    nc.compile()
    return nc


_NC_CACHE = {}


def get_nc(key, *args, **kw):
    if key not in _NC_CACHE:
        _NC_CACHE[key] = build_model_nc(*args, **kw)
    return _NC_CACHE[key]


def make_in_maps(input_ids, attention_mask, lengths, emb_in, Wq, Wk, Wv, Wo,
                 W1, W2, resweight, S, T):
    ids = np.asarray(input_ids).astype(np.int32)
    mask = np.asarray(attention_mask).astype(np.float32)
    lens = np.asarray(lengths).astype(np.float32)
    emb = np.ascontiguousarray(np.asarray(emb_in, dtype=np.float32))
    rw = np.asarray(resweight, dtype=np.float32)

    bf = ml_dtypes.bfloat16
    wq = np.ascontiguousarray(np.asarray(Wq, dtype=np.float32)).astype(bf)
    wk = np.ascontiguousarray(np.asarray(Wk, dtype=np.float32)).astype(bf)
    wv = np.ascontiguousarray(np.asarray(Wv, dtype=np.float32)).astype(bf)
    wo = np.ascontiguousarray(
        np.asarray(Wo, dtype=np.float32) * rw[:, None, None]).astype(bf)
    w1 = np.ascontiguousarray(np.asarray(W1, dtype=np.float32)).astype(bf)
    w2 = np.ascontiguousarray(
        np.asarray(W2, dtype=np.float32) * rw[:, None, None]).astype(bf)

    s_idx = np.arange(S, dtype=np.float32)
    ang = (math.pi / 2) * s_idx[None, :] / lens[:, None]
    cw, sw = np.cos(ang), np.sin(ang)
    kcw_f, ksw_f = cw * mask, sw * mask

    in_maps = []
    for c in range(N_CORES):
        b, half = c // 2, c % 2
        sl = slice(half * T, (half + 1) * T)
        cwsw = np.empty((128, T), np.float32)
        cwsw[0:64, :] = cw[b, sl][None, :]
        cwsw[64:128, :] = sw[b, sl][None, :]
        swcw = np.empty((128, T), np.float32)
        swcw[0:64, :] = sw[b, sl][None, :]
        swcw[64:128, :] = cw[b, sl][None, :]
        in_maps.append({
            "ids": np.ascontiguousarray(ids[b, sl]),
            "emb": emb,
            "wq": wq, "wk": wk, "wv": wv, "wo": wo, "w1": w1, "w2": w2,
            "cwsw": cwsw.astype(bf),
            "swcw": swcw.astype(bf),
            "kcw": np.ascontiguousarray(
                kcw_f[b, sl].reshape(T // 128, 128).T).astype(np.float32),
            "ksw": np.ascontiguousarray(
                ksw_f[b, sl].reshape(T // 128, 128).T).astype(np.float32),
        })
    return in_maps


def head_loss(res_list, emb_out, labels, B, C):
    eo = np.asarray(emb_out, dtype=np.float32)
    lab = np.asarray(labels).astype(np.int64)
    x0 = np.stack([np.asarray(res_list[2 * b]["x0"]).reshape(-1)
                   for b in range(B)])
    logits = x0 @ eo
    m = logits.max(axis=1, keepdims=True)
    logp = logits - (m + np.log(np.exp(logits - m).sum(axis=1, keepdims=True)))
    loss = np.float32(-np.mean(logp[np.arange(B), lab]))
    return loss, logits.astype(np.float32)


def kernel(input_ids, labels, attention_mask, lengths, emb_in, Wq, Wk, Wv, Wo,
           W1, W2, resweight, emb_out):
    B, S, D, H, L, V, C = 4, 2048, 1024, 16, 6, 32000, 4
    FF, T = 4 * D, 1024

    in_maps = make_in_maps(input_ids, attention_mask, lengths, emb_in,
                           Wq, Wk, Wv, Wo, W1, W2, resweight, S, T)
    nc = get_nc("full", V, D, H, L, FF, T)
    res = bass_utils.run_bass_kernel_spmd(nc, in_maps,
                                          core_ids=list(range(N_CORES)))
    return head_loss(res.results, emb_out, labels, B, C)


# revision 17
# speedup vs baseline: 1.2906x; 1.2176x over previous
"""cosFormer linear-attention transformer forward on 8 TRN2 NeuronCores.

Sharding: token-parallel. Core i handles batch b=i//2, sequence half i%2
(1024 tokens). All position-wise ops (projections, FFN) are local; the
linear-attention kv/ksum statistics are summed across the 2 cores sharing a
batch element with a pairwise AllReduce per layer (~270KB, overlapped with
the q projection).

Layout: activations are feature-major x^T [D(part), T(free)] in SBUF; k/v come
out of the PE token-major, which is exactly what the kv einsum needs as the
stationary operand, so no transposes are needed after the embedding.
Matmul inputs are bf16; PSUM accumulation and the residual stream are fp32.
Wo and W2 are pre-scaled by the ReZero resweight on the host, so residuals
are plain adds.
"""
import math
from contextlib import ExitStack

import numpy as np
import ml_dtypes

import concourse.bass as bass
import concourse.mybir as mybir
import concourse.tile as tile
from concourse import bacc, bass_utils
from concourse.masks import make_identity

F32 = mybir.dt.float32
BF16 = mybir.dt.bfloat16
F32R = mybir.dt.float32r
I32 = mybir.dt.int32
AF = mybir.ActivationFunctionType
ALU = mybir.AluOpType

N_CORES = 8
PAIRS = [[0, 1], [2, 3], [4, 5], [6, 7]]
DH = 64


def build_model_nc(V, D, H, L, FF, T, EPS=1e-5, out_full_x=False):
    """Build the per-core Bass program. T = tokens per core (== D)."""
    KD = D // 128                # feature k-tiles
    TT = T // 128                # token tiles
    NC = min(512, T)             # free-dim chunk for matmuls
    NN = T // NC                 # token chunks
    FT = FF // 128               # ffn row tiles
    FCC = min(512, FF)           # w1 staging chunk cols
    FC = FF // FCC
    FM = FCC // 128
    VA = DH + 1                  # v augmented with ones column
    assert D == T and D % 128 == 0 and FF % FCC == 0 and H * DH == D
    assert H == 2 * KD

    nc = bacc.Bacc("TRN2", target_bir_lowering=False, debug=False,
                   num_devices=N_CORES)

    ids_d = nc.dram_tensor("ids", [T], I32, kind="ExternalInput")
    emb_d = nc.dram_tensor("emb", [V, D], F32, kind="ExternalInput")
    wq_d = nc.dram_tensor("wq", [L, D, D], BF16, kind="ExternalInput")
    wk_d = nc.dram_tensor("wk", [L, D, D], BF16, kind="ExternalInput")
    wv_d = nc.dram_tensor("wv", [L, D, D], BF16, kind="ExternalInput")
    wo_d = nc.dram_tensor("wo", [L, D, D], BF16, kind="ExternalInput")
    w1_d = nc.dram_tensor("w1", [L, D, FF], BF16, kind="ExternalInput")
    w2_d = nc.dram_tensor("w2", [L, FF, D], BF16, kind="ExternalInput")
    cwsw_d = nc.dram_tensor("cwsw", [128, T], BF16, kind="ExternalInput")
    swcw_d = nc.dram_tensor("swcw", [128, T], BF16, kind="ExternalInput")
    kcw_d = nc.dram_tensor("kcw", [128, TT], F32, kind="ExternalInput")
    ksw_d = nc.dram_tensor("ksw", [128, TT], F32, kind="ExternalInput")
    x0_d = nc.dram_tensor("x0", [KD, 128], F32, kind="ExternalOutput")
    if out_full_x:
        xf_d = nc.dram_tensor("xfull", [KD, 128, T], F32,
                              kind="ExternalOutput")

    with tile.TileContext(nc) as tc, ExitStack() as ctx:
        const = ctx.enter_context(tc.tile_pool(name="const", bufs=1))
        persist = ctx.enter_context(tc.tile_pool(name="persist", bufs=1))
        small = ctx.enter_context(tc.tile_pool(name="small", bufs=2))
        wstr = ctx.enter_context(tc.tile_pool(name="wstr", bufs=3))
        drp = ctx.enter_context(tc.tile_pool(name="drp", bufs=2, space="DRAM"))

        # ---- constants ----
        cwsw = const.tile([128, T], BF16, tag="cwsw")
        nc.sync.dma_start(out=cwsw[:, :], in_=cwsw_d[:, :])
        swcw = const.tile([128, T], BF16, tag="swcw")
        nc.sync.dma_start(out=swcw[:, :], in_=swcw_d[:, :])
        kcw = const.tile([128, TT], F32, tag="kcw")
        ksw = const.tile([128, TT], F32, tag="ksw")
        nc.sync.dma_start(out=kcw[:, :], in_=kcw_d[:, :])
        nc.sync.dma_start(out=ksw[:, :], in_=ksw_d[:, :])
        ones_bf = const.tile([128, DH], BF16, tag="ones")
        nc.vector.memset(ones_bf[:, :], 1.0)
        ident = const.tile([128, 128], F32, tag="ident")
        make_identity(nc, ident)

        xT = persist.tile([128, KD, T], F32, tag="xT")
        xbf = persist.tile([128, KD, T], BF16, tag="xbf")

        # ---- embedding gather + transpose to feature-major ----
        ids_sb = const.tile([128, TT], I32, tag="ids")
        nc.sync.dma_start(
            out=ids_sb[:, :],
            in_=bass.AP(tensor=ids_d, offset=0, ap=[[1, 128], [128, TT]]),
        )
        with tc.tile_pool(name="emb", bufs=2) as ep, \
             tc.tile_pool(name="embp", bufs=3, space="PSUM") as epp:
            for t in range(TT):
                xtok = ep.tile([128, D], F32, tag="xtok")
                nc.gpsimd.indirect_dma_start(
                    out=xtok[:, :], out_offset=None,
                    in_=emb_d[:, :],
                    in_offset=bass.IndirectOffsetOnAxis(
                        ap=ids_sb[:, t:t + 1], axis=0),
                )
                for k in range(KD):
                    ps = epp.tile([128, 128], F32, tag="tp")
                    nc.tensor.transpose(ps[:, :],
                                        xtok[:, k * 128:(k + 1) * 128],
                                        ident[:, :])
                    nc.scalar.activation(xT[:, k, t * 128:(t + 1) * 128],
                                         ps[:, :], AF.Copy)
                    nc.vector.tensor_copy(xbf[:, k, t * 128:(t + 1) * 128],
                                          ps[:, :])

        # ---- layers ----
        for l in range(L):
            # ---------- k/v projections + kv aggregation ----------
            # kv accumulated in PSUM: heads grouped 7+7+2 per bank tile
            HG = [list(range(0, min(7, H)))]
            if H > 7:
                HG.append(list(range(7, min(14, H))))
            if H > 14:
                HG.append(list(range(14, H)))
            with tc.tile_pool(name=f"wkv{l}", bufs=1) as wkvp, \
                 tc.tile_pool(name=f"kvt{l}", bufs=2) as kvtp, \
                 tc.tile_pool(name=f"pkv{l}", bufs=2, space="PSUM") as pkvp, \
                 tc.tile_pool(name=f"pag{l}", bufs=1, space="PSUM") as pagp:
                pag = []
                for g, hs in enumerate(HG):
                    pg = pagp.tile([128, len(hs) * VA], F32, tag=f"pag{g}",
                                   name=f"pag{l}_{g}")
                    pag.append(pg)
                wk_s = wkvp.tile([128, KD, D], BF16, tag="wk")
                wv_s = wkvp.tile([128, KD, D], BF16, tag="wv")
                for wsb, wdr in ((wk_s, wk_d), (wv_s, wv_d)):
                    for k in range(KD):
                        nc.sync.dma_start(
                            out=wsb[:, k, :],
                            in_=bass.AP(tensor=wdr,
                                        offset=l * D * D + k * 128 * D,
                                        ap=[[D, 128], [1, D]]),
                        )
                for t in range(TT):
                    kft = kvtp.tile([128, 2 * D], BF16, tag="kft")
                    vat = kvtp.tile([128, H * VA], BF16, tag="vat")
                    va4 = vat[:, :].rearrange("p (h c) -> p h c", c=VA)
                    nc.vector.memset(va4[:, :, DH:DH + 1], 1.0)
                    kf4 = kft[:, :].rearrange("p (h s c) -> p h s c",
                                              h=H, s=2)
                    for n in range(D // NC):
                        esl = bass.ts(n, NC)
                        hpc = NC // DH
                        h0 = n * NC // DH
                        psk = pkvp.tile([128, NC], F32, tag="psk")
                        psv = pkvp.tile([128, NC], F32, tag="psv")
                        for k in range(KD):
                            lhs = xbf[:, k, t * 128:(t + 1) * 128]
                            nc.tensor.matmul(psk[:, :], lhs, wk_s[:, k, esl],
                                             start=(k == 0),
                                             stop=(k == KD - 1))
                        for k in range(KD):
                            lhs = xbf[:, k, t * 128:(t + 1) * 128]
                            nc.tensor.matmul(psv[:, :], lhs, wv_s[:, k, esl],
                                             start=(k == 0),
                                             stop=(k == KD - 1))
                        psk4 = psk[:, :].rearrange("p (h c) -> p h c", c=DH)
                        psv4 = psv[:, :].rearrange("p (h c) -> p h c", c=DH)
                        nc.scalar.activation(kf4[:, h0:h0 + hpc, 0, :],
                                             psk4[:, :, :], AF.Relu,
                                             scale=kcw[:, t:t + 1])
                        nc.scalar.activation(kf4[:, h0:h0 + hpc, 1, :],
                                             psk4[:, :, :], AF.Relu,
                                             scale=ksw[:, t:t + 1])
                        nc.scalar.activation(va4[:, h0:h0 + hpc, 0:DH],
                                             psv4[:, :, :], AF.Copy)
                    for g, hs in enumerate(HG):
                        for i, h in enumerate(hs):
                            nc.tensor.matmul(
                                pag[g][:, i * VA:(i + 1) * VA],
                                kft[:, h * 128:(h + 1) * 128],
                                vat[:, h * VA:(h + 1) * VA],
                                start=(t == 0 and i == 0),
                                stop=(t == TT - 1 and i == len(hs) - 1))

                # copy PSUM groups -> kvloc bf16 (inside kv pools scope)
                kvloc = persist.tile([128, H * VA], BF16, tag="kvloc")
                off = 0
                for g, hs in enumerate(HG):
                    w = len(hs) * VA
                    nc.vector.tensor_copy(kvloc[:, off:off + w], pag[g][:, :])
                    off += w

            # ---------- AllReduce kv stats across the pair ----------
            cc_in = drp.tile([128, H * VA], BF16, tag="ccin")
            cc_out = drp.tile([128, H * VA], BF16, tag="ccout")
            nc.sync.dma_start(out=cc_in[:, :], in_=kvloc[:, :])
            nc.gpsimd.collective_compute(
                "AllReduce", ALU.add, replica_groups=PAIRS,
                ins=[cc_in[:, :]], outs=[cc_out[:, :]])
            kvg = persist.tile([128, H, VA], BF16, tag="kvg")
            nc.sync.dma_start(out=kvg[:, :, :], in_=cc_out[:, :])

            # ---------- q projection / attention / Wo ----------
            qt = persist.tile([128, KD, T], BF16, tag="qt")
            attnT = persist.tile([128, KD, T], BF16, tag="attnT")
            with tc.tile_pool(name=f"wqo{l}", bufs=1) as wqop, \
                 tc.tile_pool(name=f"pq{l}", bufs=3, space="PSUM") as pqp:
                wq_s = wqop.tile([128, KD, D], BF16, tag="wq")
                wo_s = wqop.tile([128, KD, D], BF16, tag="wo")
                for wsb, wdr in ((wq_s, wq_d), (wo_s, wo_d)):
                    for k in range(KD):
                        nc.sync.dma_start(
                            out=wsb[:, k, :],
                            in_=bass.AP(tensor=wdr,
                                        offset=l * D * D + k * 128 * D,
                                        ap=[[D, 128], [1, D]]),
                        )
                for m in range(KD):
                    for n in range(NN):
                        nsl = bass.ts(n, NC)
                        psq = pqp.tile([128, NC], F32, tag="psq", bufs=2)
                        for k in range(KD):
                            nc.tensor.matmul(
                                psq[:, :], wq_s[:, k, m * 128:(m + 1) * 128],
                                xbf[:, k, nsl],
                                start=(k == 0), stop=(k == KD - 1))
                        nc.scalar.activation(qt[:, m, nsl], psq[:, :], AF.Relu)

                for h in range(H):
                    j, half = h // 2, h % 2
                    qh = qt[half * 64:half * 64 + 64, j, :]
                    qf = small.tile([128, T], BF16, tag="qf")
                    # SBUF tensor_tensor inputs must share a base partition:
                    # cwsw=[cw;sw], swcw=[sw;cw] so each half reads in place.
                    hsl = slice(half * 64, half * 64 + 64)
                    cwh = cwsw[hsl, :] if half == 0 else swcw[hsl, :]
                    swh = swcw[hsl, :] if half == 0 else cwsw[hsl, :]
                    nc.vector.tensor_mul(qf[0:64, :], qh, cwh)
                    nc.vector.tensor_mul(qf[64:128, :], qh, swh)
                    for n in range(NN):
                        nsl = bass.ts(n, NC)
                        pso = pqp.tile([128, NC], F32, tag="pso", bufs=2)
                        nc.tensor.matmul(pso[0:DH, :], kvg[:, h, 0:DH],
                                         qf[:, nsl], start=True, stop=True)
                        psd = pqp.tile([128, NC], F32, tag="psd", bufs=2)
                        nc.tensor.matmul(psd[0:1, :], kvg[:, h, DH:DH + 1],
                                         qf[:, nsl], start=True, stop=True)
                        # z chain at partition base 0 (custom DVE op needs it)
                        ze = small.tile([128, NC], F32, tag="ze")
                        nc.scalar.activation(ze[0:1, :], psd[0:1, :], AF.Copy,
                                             bias=EPS)
                        zf = small.tile([128, NC], F32, tag="zf")
                        nc.vector.reciprocal_approx_fast(zf[0:1, :],
                                                         ze[0:1, :])
                        zb = small.tile([128, NC], BF16, tag="zb")
                        with nc.allow_low_precision(
                                reason="z broadcast via PE needs bf16 rhs"):
                            nc.vector.tensor_copy(zb[0:1, :], zf[0:1, :])
                        psz = pqp.tile([128, NC], F32, tag="psz", bufs=2)
                        nc.tensor.matmul(psz[0:64, :], ones_bf[0:1, :],
                                         zb[0:1, :],
                                         start=True, stop=True)
                        nb = small.tile([128, NC], BF16, tag="nb")
                        nc.scalar.activation(nb[0:64, :], pso[0:64, :],
                                             AF.Copy)
                        nc.vector.tensor_mul(
                            attnT[half * 64:half * 64 + 64, j, nsl],
                            nb[0:64, :], psz[0:64, :])

                for m in range(KD):
                    for n in range(NN):
                        nsl = bass.ts(n, NC)
                        pswo = pqp.tile([128, NC], F32, tag="pso", bufs=2)
                        for k in range(KD):
                            nc.tensor.matmul(
                                pswo[:, :], wo_s[:, k, m * 128:(m + 1) * 128],
                                attnT[:, k, nsl],
                                start=(k == 0), stop=(k == KD - 1))
                        nc.vector.tensor_add(xT[:, m, nsl], xT[:, m, nsl],
                                             pswo[:, :])
                        nc.vector.tensor_copy(xbf[:, m, nsl], xT[:, m, nsl])

            # ---------- FFN ----------
            with tc.tile_pool(name=f"ffn{l}", bufs=1) as ffp, \
                 tc.tile_pool(name=f"pfn{l}", bufs=1, space="PSUM") as pfp:
                hsb = ffp.tile([128, FT, NC], BF16, tag="hsb")
                EG = KD // 2
                for n in range(NN):
                    nsl = bass.ts(n, NC)
                    for c in range(FC):
                        w1c = wstr.tile([128, KD, FCC], BF16, tag="w1c",
                                        bufs=4)
                        nc.sync.dma_start(
                            out=w1c[:, :, :],
                            in_=bass.AP(tensor=w1_d,
                                        offset=l * D * FF + c * FCC,
                                        ap=[[FF, 128], [128 * FF, KD],
                                            [1, FCC]]),
                        )
                        for m in range(FM):
                            psh = pfp.tile([128, NC], F32, tag="psh", bufs=3)
                            for k in range(KD):
                                nc.tensor.matmul(
                                    psh[:, :],
                                    w1c[:, k, m * 128:(m + 1) * 128],
                                    xbf[:, k, nsl],
                                    start=(k == 0), stop=(k == KD - 1))
                            nc.scalar.activation(hsb[:, c * FM + m, :],
                                                 psh[:, :], AF.Relu)
                    for eg in range(2):
                        po = []
                        for e in range(EG):
                            poe = pfp.tile([128, NC], F32, tag=f"po{e}",
                                           name=f"po{l}_{n}_{eg}_{e}")
                            po.append(poe)
                        for ff in range(FT):
                            w2t = wstr.tile([128, EG * 128], BF16, tag="w2t",
                                            bufs=6)
                            nc.sync.dma_start(
                                out=w2t[:, :],
                                in_=bass.AP(
                                    tensor=w2_d,
                                    offset=l * FF * D + ff * 128 * D
                                    + eg * EG * 128,
                                    ap=[[D, 128], [1, EG * 128]]),
                            )
                            for i in range(EG):
                                nc.tensor.matmul(
                                    po[i][:, :],
                                    w2t[:, i * 128:(i + 1) * 128],
                                    hsb[:, ff, :],
                                    start=(ff == 0), stop=(ff == FT - 1))
                        for i in range(EG):
                            e = eg * EG + i
                            nc.vector.tensor_add(xT[:, e, nsl], xT[:, e, nsl],
                                                 po[i][:, :])
                            nc.vector.tensor_copy(xbf[:, e, nsl],
                                                  xT[:, e, nsl])

        # ---- outputs ----
        for k in range(KD):
            nc.sync.dma_start(out=x0_d[k, :], in_=xT[:, k, 0:1])
        if out_full_x:
            for k in range(KD):
                nc.sync.dma_start(out=xf_d[k, :, :], in_=xT[:, k, :])

    nc.compile()
    return nc


_NC_CACHE = {}


def get_nc(key, *args, **kw):
    if key not in _NC_CACHE:
        _NC_CACHE[key] = build_model_nc(*args, **kw)
    return _NC_CACHE[key]


def make_in_maps(input_ids, attention_mask, lengths, emb_in, Wq, Wk, Wv, Wo,
                 W1, W2, resweight, S, T):
    ids = np.asarray(input_ids).astype(np.int32)
    mask = np.asarray(attention_mask).astype(np.float32)
    lens = np.asarray(lengths).astype(np.float32)
    emb = np.ascontiguousarray(np.asarray(emb_in, dtype=np.float32))
    rw = np.asarray(resweight, dtype=np.float32)

    bf = ml_dtypes.bfloat16
    wq = np.ascontiguousarray(np.asarray(Wq, dtype=np.float32)).astype(bf)
    wk = np.ascontiguousarray(np.asarray(Wk, dtype=np.float32)).astype(bf)
    wv = np.ascontiguousarray(np.asarray(Wv, dtype=np.float32)).astype(bf)
    wo = np.ascontiguousarray(
        np.asarray(Wo, dtype=np.float32) * rw[:, None, None]).astype(bf)
    w1 = np.ascontiguousarray(np.asarray(W1, dtype=np.float32)).astype(bf)
    w2 = np.ascontiguousarray(
        np.asarray(W2, dtype=np.float32) * rw[:, None, None]).astype(bf)

    s_idx = np.arange(S, dtype=np.float32)
    ang = (math.pi / 2) * s_idx[None, :] / lens[:, None]
    cw, sw = np.cos(ang), np.sin(ang)
    kcw_f, ksw_f = cw * mask, sw * mask

    in_maps = []
    for c in range(N_CORES):
        b, half = c // 2, c % 2
        sl = slice(half * T, (half + 1) * T)
        cwsw = np.empty((128, T), np.float32)
        cwsw[0:64, :] = cw[b, sl][None, :]
        cwsw[64:128, :] = sw[b, sl][None, :]
        swcw = np.empty((128, T), np.float32)
        swcw[0:64, :] = sw[b, sl][None, :]
        swcw[64:128, :] = cw[b, sl][None, :]
        in_maps.append({
            "ids": np.ascontiguousarray(ids[b, sl]),
            "emb": emb,
            "wq": wq, "wk": wk, "wv": wv, "wo": wo, "w1": w1, "w2": w2,
            "cwsw": cwsw.astype(bf),
            "swcw": swcw.astype(bf),
            "kcw": np.ascontiguousarray(
                kcw_f[b, sl].reshape(T // 128, 128).T).astype(np.float32),
            "ksw": np.ascontiguousarray(
                ksw_f[b, sl].reshape(T // 128, 128).T).astype(np.float32),
        })
    return in_maps


def head_loss(res_list, emb_out, labels, B, C):
    eo = np.asarray(emb_out, dtype=np.float32)
    lab = np.asarray(labels).astype(np.int64)
    x0 = np.stack([np.asarray(res_list[2 * b]["x0"]).reshape(-1)
                   for b in range(B)])
    logits = x0 @ eo
    m = logits.max(axis=1, keepdims=True)
    logp = logits - (m + np.log(np.exp(logits - m).sum(axis=1, keepdims=True)))
    loss = np.float32(-np.mean(logp[np.arange(B), lab]))
    return loss, logits.astype(np.float32)


def kernel(input_ids, labels, attention_mask, lengths, emb_in, Wq, Wk, Wv, Wo,
           W1, W2, resweight, emb_out):
    B, S, D, H, L, V, C = 4, 2048, 1024, 16, 6, 32000, 4
    FF, T = 4 * D, 1024

    in_maps = make_in_maps(input_ids, attention_mask, lengths, emb_in,
                           Wq, Wk, Wv, Wo, W1, W2, resweight, S, T)
    nc = get_nc("full", V, D, H, L, FF, T)
    res = bass_utils.run_bass_kernel_spmd(nc, in_maps,
                                          core_ids=list(range(N_CORES)))
    return head_loss(res.results, emb_out, labels, B, C)
